# revision 25
# baseline (speedup 1.0000x reference)
"""Distributed single-head attention for Trainium2 (8 NeuronCores).

Problem: B=4, S=2048, D=1024 fp32 attention:
    q = x@Wq+bq; k = x@Wk+bk; v = x@Wv+bv
    out = softmax(q k^T / sqrt(D) + mask) v

Sharding: data-parallel over (batch, query-half): core c handles batch
c//2, query rows [1024*(c%2), 1024*(c%2)+1024). Each core projects K/V
only for its own 1024 rows; the other half comes from its pair core via
an in-pair AllGather (replica groups [[0,1],[2,3],[4,5],[6,7]]), overlapped
behind the Q projection / V projection matmuls.

Per-core host-prepared inputs:
  xt  bf16 [1024(d), 1024(s_own)]: own x rows, transposed.
  wq/wk/wv bf16 [1024(d), 1024(e)]: natural lhsT for out[e,s] matmuls.
  bq2/bk2  f32 [128, 8]: bias chunk e at [:, e] (per-partition bias).
  bvr bf16 [1, 1024]: V bias as a row (added via rank-1 matmul).
  maskp f32 [1024, 2048] (only when mask is nonzero): additive mask for
      this core's q rows, pre-divided by SCALE so the fused exp(SCALE*x)
      picks it up exactly.

On-chip per core:
  kT_own[e,1024] = Wk^T xT (+bk) -> DRAM bounce -> AllGather -> kT[e,2048]
  V_own[1024,e]  = xT^T Wv (+bv) -> DRAM bounce -> AllGather -> V[2048,e]
  qT[e,1024]     = Wq^T xT (+bq)
  per q-chunk (128 rows):
    scores[q,s] = qT^T kT  (fp32 PSUM, two 1024-wide halves)
    e = exp(SCALE*scores (+mask)), row-sums via ScalarE accum_out
    attnT = one 3D xbar DMA-transpose (bf16)
    o[q,e] = attnT^T V (fp32 PSUM), evicted with *1/rowsum fused

Key order in kT/V columns is the gather order (core pair rank 0's keys
then rank 1's) — identical for both cores and consistent between kT and
V, and softmax+PV are invariant to key permutation. With a nonzero mask
the gather order equals the natural row order, so mask columns line up.
"""

from contextlib import ExitStack

import numpy as np
import ml_dtypes

import concourse.bass as bass
import concourse.tile as tile
import concourse.mybir as mybir
from concourse import bacc
from concourse.bass_utils import run_bass_kernel_spmd

BF16 = mybir.dt.bfloat16
F32 = mybir.dt.float32
AF = mybir.ActivationFunctionType

D = 1024  # model dim (= contraction dim for projections)
S = 2048  # full sequence (keys)
Q = 1024  # queries per core
P = 128  # partitions
ND = D // P  # 8 d-chunks
NS = S // P  # 16 key chunks
NQ = Q // P  # 8 query chunks
SCALE = 1.0 / float(np.sqrt(np.float32(D)))
PAIRS = [[0, 1], [2, 3], [4, 5], [6, 7]]

_NC_CACHE: dict[bool, bacc.Bacc] = {}


def _build(use_mask: bool) -> bacc.Bacc:
    nc = bacc.Bacc("TRN2", target_bir_lowering=False, debug=False, num_devices=8)

    xt_d = nc.dram_tensor("xt", [D, Q], BF16, kind="ExternalInput")
    wq_d = nc.dram_tensor("wq", [D, D], BF16, kind="ExternalInput")
    wk_d = nc.dram_tensor("wk", [D, D], BF16, kind="ExternalInput")
    wv_d = nc.dram_tensor("wv", [D, D], BF16, kind="ExternalInput")
    bq_d = nc.dram_tensor("bq2", [P, ND], F32, kind="ExternalInput")
    bk_d = nc.dram_tensor("bk2", [P, ND], F32, kind="ExternalInput")
    bv_d = nc.dram_tensor("bvr", [1, D], BF16, kind="ExternalInput")
    if use_mask:
        mask_d = nc.dram_tensor("maskp", [Q, S], F32, kind="ExternalInput")
    out_d = nc.dram_tensor("out", [Q, D], F32, kind="ExternalOutput")

    # DRAM bounce buffers for the in-pair K/V AllGathers (split in halves so
    # each gather can start as soon as its 4-chunk block is projected)
    H = Q // 2
    ktb_in = [nc.dram_tensor(f"ktb_in{i}", [H, D], BF16) for i in range(2)]
    ktb_out = [nc.dram_tensor(f"ktb_out{i}", [Q, D], BF16) for i in range(2)]
    vtb_in = [nc.dram_tensor(f"vtb_in{i}", [H, D], BF16) for i in range(2)]
    vtb_out = [nc.dram_tensor(f"vtb_out{i}", [Q, D], BF16) for i in range(2)]

    nbuf = 1 if use_mask else 2

    with tile.TileContext(nc) as tc, ExitStack() as ctx:
        xt_pool = ctx.enter_context(tc.tile_pool(name="xt", bufs=ND))
        wq_pool = ctx.enter_context(tc.tile_pool(name="wq", bufs=ND))
        wk_pool = ctx.enter_context(tc.tile_pool(name="wk", bufs=ND))
        wv_pool = ctx.enter_context(tc.tile_pool(name="wv", bufs=ND))
        qt_pool = ctx.enter_context(tc.tile_pool(name="qt", bufs=NQ))
        kt_pool = ctx.enter_context(tc.tile_pool(name="kt", bufs=ND))
        ko_pool = ctx.enter_context(tc.tile_pool(name="ko", bufs=3))
        vo_pool = ctx.enter_context(tc.tile_pool(name="vo", bufs=4))
        vt_pool = ctx.enter_context(tc.tile_pool(name="vt", bufs=NS))
        const_pool = ctx.enter_context(tc.tile_pool(name="const", bufs=1))
        exp_pool = ctx.enter_context(tc.tile_pool(name="exp", bufs=nbuf))
        at_pool = ctx.enter_context(tc.tile_pool(name="at", bufs=nbuf))
        stat_pool = ctx.enter_context(tc.tile_pool(name="stat", bufs=4 * nbuf))
        o_pool = ctx.enter_context(tc.tile_pool(name="o", bufs=2))
        if use_mask:
            m_pool = ctx.enter_context(tc.tile_pool(name="m", bufs=2))
        psum = ctx.enter_context(tc.tile_pool(name="psum", bufs=4, space="PSUM"))

        # ---- loads (wk first: kT is the first compute phase) ----
        xt = [xt_pool.tile([P, Q], BF16, tag="xt", name=f"xt{i}") for i in range(ND)]
        wq = [wq_pool.tile([P, D], BF16, tag="wq", name=f"wq{i}") for i in range(ND)]
        wk = [wk_pool.tile([P, D], BF16, tag="wk", name=f"wk{i}") for i in range(ND)]
        wv = [wv_pool.tile([P, D], BF16, tag="wv", name=f"wv{i}") for i in range(ND)]
        for d in range(ND):
            nc.sync.dma_start(xt[d][:], xt_d[d * P : (d + 1) * P, :])
            nc.sync.dma_start(wk[d][:], wk_d[d * P : (d + 1) * P, :])
        for d in range(ND):
            nc.gpsimd.dma_start(wv[d][:], wv_d[d * P : (d + 1) * P, :])
        for d in range(ND):
            nc.sync.dma_start(wq[d][:], wq_d[d * P : (d + 1) * P, :])
        bq_sb = const_pool.tile([P, ND], F32, tag="bq")
        bk_sb = const_pool.tile([P, ND], F32, tag="bk")
        bv_sb = const_pool.tile([1, D], BF16, tag="bv")
        ones_sb = const_pool.tile([1, P], BF16, tag="ones")
        nc.scalar.dma_start(bq_sb[:], bq_d[:, :])
        nc.scalar.dma_start(bk_sb[:], bk_d[:, :])
        nc.scalar.dma_start(bv_sb[:], bv_d[:, :])
        nc.gpsimd.memset(ones_sb[:], 1.0)

        def proj_block(w, bias_sb, eb, pool, tg):
            """4 e-chunks at once, d-outer so PE consumes w/xt tiles as they
            arrive. Returns 4 evicted bf16 [P, Q] tiles."""
            pss = [
                psum.tile([P, Q], F32, tag="ps", name=f"ps{tg}{eb}_{j}")
                for j in range(4)
            ]
            for d in range(ND):
                for j in range(4):
                    e = eb * 4 + j
                    for n in range(2):
                        nc.tensor.matmul(
                            pss[j][:, n * 512 : (n + 1) * 512],
                            lhsT=w[d][:, e * P : (e + 1) * P],
                            rhs=xt[d][:, n * 512 : (n + 1) * 512],
                            start=(d == 0),
                            stop=(d == ND - 1),
                        )
            outs = []
            for j in range(4):
                e = eb * 4 + j
                t = pool.tile([P, Q], BF16, tag=tg, name=f"{tg}{e}")
                if j % 2 == 0:
                    nc.scalar.activation(
                        t[:], pss[j][:], AF.Identity, bias=bias_sb[:, e : e + 1]
                    )
                else:
                    nc.vector.tensor_scalar_add(t[:], pss[j][:], bias_sb[:, e : e + 1])
                outs.append(t)
            return outs

        # ---- projections: kT block0, V block0, V block1, kT block1 so both
        # AllGathers complete well before the attention loop needs them ----
        kt = [kt_pool.tile([P, S], BF16, tag="kt", name=f"kt{i}") for i in range(ND)]
        vt = [vt_pool.tile([P, D], BF16, tag="vt", name=f"vt{i}") for i in range(NS)]

        def kt_block(eb):
            for j, t in enumerate(proj_block(wk, bk_sb, eb, ko_pool, "ko")):
                nc.sync.dma_start(ktb_in[eb][j * P : (j + 1) * P, :], t[:])
            nc.gpsimd.collective_compute(
                "AllGather",
                mybir.AluOpType.bypass,
                replica_groups=PAIRS,
                ins=[ktb_in[eb].ap().opt()],
                outs=[ktb_out[eb].ap().opt()],
            )
            # one 3D DMA per e-chunk: cols 0:Q <- pair-rank0 rows, Q:S <- rank1
            src_r = ktb_out[eb].ap().rearrange("(h r) k -> r h k", h=2)
            for j in range(4):
                e = eb * 4 + j
                nc.gpsimd.dma_start(
                    kt[e].rearrange("p (h k) -> p h k", h=2),
                    src_r[j * P : (j + 1) * P],
                )

        def v_block(kb):
            pss = [
                psum.tile([P, D], F32, tag="ps", name=f"vps{kb}_{j}")
                for j in range(4)
            ]
            for d in range(ND):
                for j in range(4):
                    k = kb * 4 + j
                    for n in range(2):
                        nc.tensor.matmul(
                            pss[j][:, n * 512 : (n + 1) * 512],
                            lhsT=xt[d][:, k * P : (k + 1) * P],
                            rhs=wv[d][:, n * 512 : (n + 1) * 512],
                            start=(d == 0),
                            stop=False,
                        )
            for j in range(4):
                for n in range(2):
                    nc.tensor.matmul(
                        pss[j][:, n * 512 : (n + 1) * 512],
                        lhsT=ones_sb[:, 0:P],
                        rhs=bv_sb[:, n * 512 : (n + 1) * 512],
                        start=False,
                        stop=True,
                    )
            for j in range(4):
                k = kb * 4 + j
                t = vo_pool.tile([P, D], BF16, tag="vo", name=f"vo{k}")
                if j % 2 == 0:
                    nc.vector.tensor_copy(t[:], pss[j][:])
                else:
                    nc.scalar.copy(t[:], pss[j][:])
                nc.scalar.dma_start(vtb_in[kb][j * P : (j + 1) * P, :], t[:])
            nc.gpsimd.collective_compute(
                "AllGather",
                mybir.AluOpType.bypass,
                replica_groups=PAIRS,
                ins=[vtb_in[kb].ap().opt()],
                outs=[vtb_out[kb].ap().opt()],
            )
            # block kb gathered global k-chunks {kb*4..} (rank0), {8+kb*4..} (rank1)
            for j in range(4):
                nc.gpsimd.dma_start(
                    vt[kb * 4 + j][:], vtb_out[kb][j * P : (j + 1) * P, :]
                )
                nc.gpsimd.dma_start(
                    vt[8 + kb * 4 + j][:],
                    vtb_out[kb][H + j * P : H + (j + 1) * P, :],
                )

        kt_block(0)
        v_block(0)
        v_block(1)
        kt_block(1)

        # ---- qT[e,q] ----
        qt = []
        for eb in range(2):
            qt.extend(proj_block(wq, bq_sb, eb, qt_pool, "qt"))

        # ---- attention, software-pipelined over 8 q-chunks ----
        def scores_phase(qc):
            """scores matmuls + exp(+mask) + row sums for q-chunk qc."""
            exp_sb = exp_pool.tile([P, S], BF16, tag="exp", name=f"exp{qc}")
            sums = stat_pool.tile([P, 2], F32, tag="sums", name=f"sums{qc}")
            for half in range(2):
                ps = psum.tile([P, Q], F32, tag="ps", name=f"sps{qc}_{half}")
                for e in range(ND):
                    for n in range(2):
                        nc.tensor.matmul(
                            ps[:, n * 512 : (n + 1) * 512],
                            lhsT=qt[e][:, qc * P : (qc + 1) * P],
                            rhs=kt[e][:, half * 1024 + n * 512 : half * 1024 + (n + 1) * 512],
                            start=(e == 0),
                            stop=(e == ND - 1),
                        )
                if use_mask:
                    mt = m_pool.tile([P, Q], F32, tag="m", name=f"mt{qc}_{half}")
                    nc.sync.dma_start(
                        mt[:], mask_d[qc * P : (qc + 1) * P, half * 1024 : (half + 1) * 1024]
                    )
                    nc.vector.tensor_add(ps[:], ps[:], mt[:])
                nc.scalar.activation(
                    exp_sb[:, half * 1024 : (half + 1) * 1024],
                    ps[:],
                    AF.Exp,
                    scale=SCALE,
                    accum_out=sums[:, half : half + 1],
                )
            return exp_sb, sums

        def pv_phase(qc, exp_sb, sums):
            """transpose + PV + normalized eviction for q-chunk qc."""
            rsum = stat_pool.tile([P, 1], F32, tag="rsum", name=f"rsum{qc}")
            nc.vector.tensor_add(rsum[:], sums[:, 0:1], sums[:, 1:2])
            rinv = stat_pool.tile([P, 1], F32, tag="rinv", name=f"rinv{qc}")
            nc.vector.reciprocal(rinv[:], rsum[:])
            at_sb = at_pool.tile([P, S], BF16, tag="at", name=f"at{qc}")
            # one xbar transpose for all 16 chunks: out[p, c, q] = exp[q, c*128+p]
            nc.scalar.dma_start(
                out=at_sb.rearrange("p (c q) -> p c q", q=P),
                in_=exp_sb[:, :],
                transpose=True,
            )
            pv = psum.tile([P, D], F32, tag="ps", name=f"pv{qc}")
            pv_order = [0, 1, 2, 3, 8, 9, 10, 11, 4, 5, 6, 7, 12, 13, 14, 15]
            for i, k in enumerate(pv_order):
                for n in range(2):
                    nc.tensor.matmul(
                        pv[:, n * 512 : (n + 1) * 512],
                        lhsT=at_sb[:, k * P : (k + 1) * P],
                        rhs=vt[k][:, n * 512 : (n + 1) * 512],
                        start=(i == 0),
                        stop=(i == NS - 1),
                    )
            ot = o_pool.tile([P, D], F32, tag="o", name=f"ot{qc}")
            nc.vector.tensor_scalar_mul(ot[:], pv[:], rinv[:])
            nc.sync.dma_start(out_d[qc * P : (qc + 1) * P, :], ot[:])

        # emit scores(qc+1) before pv(qc) so the PE never stalls waiting on
        # the exp/transpose of the current chunk
        pend = scores_phase(0)
        for qc in range(NQ):
            nxt = scores_phase(qc + 1) if qc + 1 < NQ else None
            pv_phase(qc, *pend)
            pend = nxt

    nc.compile()
    return nc


def _get_nc(use_mask: bool) -> bacc.Bacc:
    if use_mask not in _NC_CACHE:
        _NC_CACHE[use_mask] = _build(use_mask)
    return _NC_CACHE[use_mask]


def kernel(x, mask, Wq, bq, Wk, bk, Wv, bv):
    x = np.asarray(x, dtype=np.float32)
    mask = np.asarray(mask, dtype=np.float32)
    Wq = np.asarray(Wq, dtype=np.float32)
    bq = np.asarray(bq, dtype=np.float32)
    Wk = np.asarray(Wk, dtype=np.float32)
    bk = np.asarray(bk, dtype=np.float32)
    Wv = np.asarray(Wv, dtype=np.float32)
    bv = np.asarray(bv, dtype=np.float32)

    B = x.shape[0]
    use_mask = bool(np.any(mask))
    nc = _get_nc(use_mask)

    bf = ml_dtypes.bfloat16
    wq_b = Wq.astype(bf)
    wk_b = Wk.astype(bf)
    wv_b = Wv.astype(bf)
    bq2 = np.ascontiguousarray(bq.reshape(ND, P).T)
    bk2 = np.ascontiguousarray(bk.reshape(ND, P).T)
    bvr = bv.reshape(1, D).astype(bf)

    in_maps = []
    for c in range(8):
        b, h = divmod(c, 2)
        off = h * Q
        xt = np.ascontiguousarray(x[b, off : off + Q].T).astype(bf)
        im = {
            "xt": xt,
            "wq": wq_b,
            "wk": wk_b,
            "wv": wv_b,
            "bq2": bq2,
            "bk2": bk2,
            "bvr": bvr,
        }
        if use_mask:
            im["maskp"] = np.ascontiguousarray(
                mask[off : off + Q] / np.float32(SCALE)
            ).astype(np.float32)
        in_maps.append(im)

    res = run_bass_kernel_spmd(nc, in_maps, core_ids=list(range(8)))

    out = np.empty((B, S, D), dtype=np.float32)
    for c in range(8):
        b, h = divmod(c, 2)
        out[b, h * Q : (h + 1) * Q, :] = res.results[c]["out"]
    return out


# revision 26
# speedup vs baseline: 1.0534x; 1.0534x over previous
"""Distributed single-head attention for Trainium2 (8 NeuronCores).

Problem: B=4, S=2048, D=1024 fp32 attention:
    q = x@Wq+bq; k = x@Wk+bk; v = x@Wv+bv
    out = softmax(q k^T / sqrt(D) + mask) v

Sharding: data-parallel over (batch, query-half): core c handles batch
c//2, query rows [1024*(c%2), 1024*(c%2)+1024). Each core projects K/V
only for its own 1024 rows; the other half comes from its pair core via
an in-pair AllGather (replica groups [[0,1],[2,3],[4,5],[6,7]]), overlapped
behind the Q projection / V projection matmuls.

Per-core host-prepared inputs:
  xt  bf16 [1024(d), 1024(s_own)]: own x rows, transposed.
  wq/wk/wv bf16 [1024(d), 1024(e)]: natural lhsT for out[e,s] matmuls.
  bq2/bk2  f32 [128, 8]: bias chunk e at [:, e] (per-partition bias).
  bvr bf16 [1, 1024]: V bias as a row (added via rank-1 matmul).
  maskp f32 [1024, 2048] (only when mask is nonzero): additive mask for
      this core's q rows, pre-divided by SCALE so the fused exp(SCALE*x)
      picks it up exactly.

On-chip per core:
  kT_own[e,1024] = Wk^T xT (+bk) -> DRAM bounce -> AllGather -> kT[e,2048]
  V_own[1024,e]  = xT^T Wv (+bv) -> DRAM bounce -> AllGather -> V[2048,e]
  qT[e,1024]     = Wq^T xT (+bq)
  per q-chunk (128 rows):
    scores[q,s] = qT^T kT  (fp32 PSUM, two 1024-wide halves)
    e = exp(SCALE*scores (+mask)), row-sums via ScalarE accum_out
    attnT = one 3D xbar DMA-transpose (bf16)
    o[q,e] = attnT^T V (fp32 PSUM), evicted with *1/rowsum fused

Key order in kT/V columns is the gather order (core pair rank 0's keys
then rank 1's) — identical for both cores and consistent between kT and
V, and softmax+PV are invariant to key permutation. With a nonzero mask
the gather order equals the natural row order, so mask columns line up.
"""

from contextlib import ExitStack

import numpy as np
import ml_dtypes

import concourse.bass as bass
import concourse.tile as tile
import concourse.mybir as mybir
from concourse import bacc
from concourse.bass_utils import run_bass_kernel_spmd

BF16 = mybir.dt.bfloat16
F32 = mybir.dt.float32
AF = mybir.ActivationFunctionType

D = 1024  # model dim (= contraction dim for projections)
S = 2048  # full sequence (keys)
Q = 1024  # queries per core
P = 128  # partitions
ND = D // P  # 8 d-chunks
NS = S // P  # 16 key chunks
NQ = Q // P  # 8 query chunks
SCALE = 1.0 / float(np.sqrt(np.float32(D)))
PAIRS = [[0, 1], [2, 3], [4, 5], [6, 7]]

_NC_CACHE: dict[bool, bacc.Bacc] = {}


def _build(use_mask: bool) -> bacc.Bacc:
    nc = bacc.Bacc("TRN2", target_bir_lowering=False, debug=False, num_devices=8)

    xt_d = nc.dram_tensor("xt", [D, Q], BF16, kind="ExternalInput")
    wq_d = nc.dram_tensor("wq", [D, D], BF16, kind="ExternalInput")
    wk_d = nc.dram_tensor("wk", [D, D], BF16, kind="ExternalInput")
    wv_d = nc.dram_tensor("wv", [D, D], BF16, kind="ExternalInput")
    bq_d = nc.dram_tensor("bq2", [P, ND], F32, kind="ExternalInput")
    bk_d = nc.dram_tensor("bk2", [P, ND], F32, kind="ExternalInput")
    bv_d = nc.dram_tensor("bvr", [1, D], BF16, kind="ExternalInput")
    if use_mask:
        mask_d = nc.dram_tensor("maskp", [Q, S], F32, kind="ExternalInput")
    out_d = nc.dram_tensor("out", [Q, D], F32, kind="ExternalOutput")

    # DRAM bounce buffers for the in-pair K/V AllGathers (split in halves so
    # each gather can start as soon as its 4-chunk block is projected)
    H = Q // 2
    ktb_in = [nc.dram_tensor(f"ktb_in{i}", [H, D], BF16) for i in range(2)]
    ktb_out = [nc.dram_tensor(f"ktb_out{i}", [Q, D], BF16) for i in range(2)]
    vtb_in = [nc.dram_tensor(f"vtb_in{i}", [H, D], BF16) for i in range(2)]
    vtb_out = [nc.dram_tensor(f"vtb_out{i}", [Q, D], BF16) for i in range(2)]

    nbuf = 1 if use_mask else 2

    with tile.TileContext(nc) as tc, ExitStack() as ctx:
        xt_pool = ctx.enter_context(tc.tile_pool(name="xt", bufs=ND))
        wq_pool = ctx.enter_context(tc.tile_pool(name="wq", bufs=ND))
        wk_pool = ctx.enter_context(tc.tile_pool(name="wk", bufs=ND))
        wv_pool = ctx.enter_context(tc.tile_pool(name="wv", bufs=ND))
        qt_pool = ctx.enter_context(tc.tile_pool(name="qt", bufs=NQ))
        kt_pool = ctx.enter_context(tc.tile_pool(name="kt", bufs=ND))
        ko_pool = ctx.enter_context(tc.tile_pool(name="ko", bufs=3))
        vo_pool = ctx.enter_context(tc.tile_pool(name="vo", bufs=4))
        vt_pool = ctx.enter_context(tc.tile_pool(name="vt", bufs=NS))
        const_pool = ctx.enter_context(tc.tile_pool(name="const", bufs=1))
        exp_pool = ctx.enter_context(tc.tile_pool(name="exp", bufs=nbuf))
        at_pool = ctx.enter_context(tc.tile_pool(name="at", bufs=nbuf))
        stat_pool = ctx.enter_context(tc.tile_pool(name="stat", bufs=4 * nbuf))
        o_pool = ctx.enter_context(tc.tile_pool(name="o", bufs=2))
        if use_mask:
            m_pool = ctx.enter_context(tc.tile_pool(name="m", bufs=2))
        psum = ctx.enter_context(tc.tile_pool(name="psum", bufs=4, space="PSUM"))

        # ---- loads (wk first: kT is the first compute phase) ----
        xt = [xt_pool.tile([P, Q], BF16, tag="xt", name=f"xt{i}") for i in range(ND)]
        wq = [wq_pool.tile([P, D], BF16, tag="wq", name=f"wq{i}") for i in range(ND)]
        wk = [wk_pool.tile([P, D], BF16, tag="wk", name=f"wk{i}") for i in range(ND)]
        wv = [wv_pool.tile([P, D], BF16, tag="wv", name=f"wv{i}") for i in range(ND)]
        for d in range(ND):
            nc.sync.dma_start(xt[d][:], xt_d[d * P : (d + 1) * P, :])
            nc.sync.dma_start(wk[d][:], wk_d[d * P : (d + 1) * P, :])
        for d in range(ND):
            nc.gpsimd.dma_start(wv[d][:], wv_d[d * P : (d + 1) * P, :])
        for d in range(ND):
            nc.sync.dma_start(wq[d][:], wq_d[d * P : (d + 1) * P, :])
        bq_sb = const_pool.tile([P, ND], F32, tag="bq")
        bk_sb = const_pool.tile([P, ND], F32, tag="bk")
        bv_sb = const_pool.tile([1, D], BF16, tag="bv")
        ones_sb = const_pool.tile([1, P], BF16, tag="ones")
        nc.scalar.dma_start(bq_sb[:], bq_d[:, :])
        nc.scalar.dma_start(bk_sb[:], bk_d[:, :])
        nc.scalar.dma_start(bv_sb[:], bv_d[:, :])
        nc.gpsimd.memset(ones_sb[:], 1.0)

        def proj_block(w, bias_sb, eb, pool, tg):
            """4 e-chunks at once, d-outer so PE consumes w/xt tiles as they
            arrive. Returns 4 evicted bf16 [P, Q] tiles."""
            pss = [
                psum.tile([P, Q], F32, tag="ps", name=f"ps{tg}{eb}_{j}")
                for j in range(4)
            ]
            for d in range(ND):
                for j in range(4):
                    e = eb * 4 + j
                    for n in range(2):
                        nc.tensor.matmul(
                            pss[j][:, n * 512 : (n + 1) * 512],
                            lhsT=w[d][:, e * P : (e + 1) * P],
                            rhs=xt[d][:, n * 512 : (n + 1) * 512],
                            start=(d == 0),
                            stop=(d == ND - 1),
                        )
            outs = []
            for j in range(4):
                e = eb * 4 + j
                t = pool.tile([P, Q], BF16, tag=tg, name=f"{tg}{e}")
                if j % 2 == 0:
                    nc.scalar.activation(
                        t[:], pss[j][:], AF.Identity, bias=bias_sb[:, e : e + 1]
                    )
                else:
                    nc.vector.tensor_scalar_add(t[:], pss[j][:], bias_sb[:, e : e + 1])
                outs.append(t)
            return outs

        # ---- kT_own + pipelined bounce/gather/scatter ----
        kt = [kt_pool.tile([P, S], BF16, tag="kt", name=f"kt{i}") for i in range(ND)]
        for eb in range(2):
            for j, t in enumerate(proj_block(wk, bk_sb, eb, ko_pool, "ko")):
                nc.sync.dma_start(ktb_in[eb][j * P : (j + 1) * P, :], t[:])
            nc.gpsimd.collective_compute(
                "AllGather",
                mybir.AluOpType.bypass,
                replica_groups=PAIRS,
                ins=[ktb_in[eb].ap().opt()],
                outs=[ktb_out[eb].ap().opt()],
            )
        for eb in range(2):
            # one 3D DMA per e-chunk: cols 0:Q <- pair-rank0 rows, Q:S <- rank1
            src_r = ktb_out[eb].ap().rearrange("(h r) k -> r h k", h=2)
            for j in range(4):
                e = eb * 4 + j
                nc.gpsimd.dma_start(
                    kt[e].rearrange("p (h k) -> p h k", h=2),
                    src_r[j * P : (j + 1) * P],
                )

        # ---- V_own[k,e] = xT^T Wv + bv ; pipelined bounce/gather/scatter ----
        vt = [vt_pool.tile([P, D], BF16, tag="vt", name=f"vt{i}") for i in range(NS)]
        for kb in range(2):
            pss = [
                psum.tile([P, D], F32, tag="ps", name=f"vps{kb}_{j}")
                for j in range(4)
            ]
            for d in range(ND):
                for j in range(4):
                    k = kb * 4 + j
                    for n in range(2):
                        nc.tensor.matmul(
                            pss[j][:, n * 512 : (n + 1) * 512],
                            lhsT=xt[d][:, k * P : (k + 1) * P],
                            rhs=wv[d][:, n * 512 : (n + 1) * 512],
                            start=(d == 0),
                            stop=False,
                        )
            for j in range(4):
                for n in range(2):
                    nc.tensor.matmul(
                        pss[j][:, n * 512 : (n + 1) * 512],
                        lhsT=ones_sb[:, 0:P],
                        rhs=bv_sb[:, n * 512 : (n + 1) * 512],
                        start=False,
                        stop=True,
                    )
            for j in range(4):
                k = kb * 4 + j
                t = vo_pool.tile([P, D], BF16, tag="vo", name=f"vo{k}")
                if j % 2 == 0:
                    nc.vector.tensor_copy(t[:], pss[j][:])
                else:
                    nc.scalar.copy(t[:], pss[j][:])
                nc.scalar.dma_start(vtb_in[kb][j * P : (j + 1) * P, :], t[:])
            nc.gpsimd.collective_compute(
                "AllGather",
                mybir.AluOpType.bypass,
                replica_groups=PAIRS,
                ins=[vtb_in[kb].ap().opt()],
                outs=[vtb_out[kb].ap().opt()],
            )
            # block kb gathered global k-chunks {kb*4..} (rank0), {8+kb*4..} (rank1)
            for j in range(4):
                nc.gpsimd.dma_start(
                    vt[kb * 4 + j][:], vtb_out[kb][j * P : (j + 1) * P, :]
                )
                nc.gpsimd.dma_start(
                    vt[8 + kb * 4 + j][:],
                    vtb_out[kb][H + j * P : H + (j + 1) * P, :],
                )

        # ---- qT[e,q] ----
        qt = []
        for eb in range(2):
            qt.extend(proj_block(wq, bq_sb, eb, qt_pool, "qt"))

        # ---- attention, software-pipelined over 8 q-chunks ----
        def scores_phase(qc):
            """scores matmuls + exp(+mask) + row sums for q-chunk qc."""
            exp_sb = exp_pool.tile([P, S], BF16, tag="exp", name=f"exp{qc}")
            sums = stat_pool.tile([P, 2], F32, tag="sums", name=f"sums{qc}")
            for half in range(2):
                ps = psum.tile([P, Q], F32, tag="ps", name=f"sps{qc}_{half}")
                for e in range(ND):
                    for n in range(2):
                        nc.tensor.matmul(
                            ps[:, n * 512 : (n + 1) * 512],
                            lhsT=qt[e][:, qc * P : (qc + 1) * P],
                            rhs=kt[e][:, half * 1024 + n * 512 : half * 1024 + (n + 1) * 512],
                            start=(e == 0),
                            stop=(e == ND - 1),
                        )
                if use_mask:
                    mt = m_pool.tile([P, Q], F32, tag="m", name=f"mt{qc}_{half}")
                    nc.sync.dma_start(
                        mt[:], mask_d[qc * P : (qc + 1) * P, half * 1024 : (half + 1) * 1024]
                    )
                    nc.vector.tensor_add(ps[:], ps[:], mt[:])
                nc.scalar.activation(
                    exp_sb[:, half * 1024 : (half + 1) * 1024],
                    ps[:],
                    AF.Exp,
                    scale=SCALE,
                    accum_out=sums[:, half : half + 1],
                )
            return exp_sb, sums

        def pv_phase(qc, exp_sb, sums):
            """transpose + PV + normalized eviction for q-chunk qc."""
            rsum = stat_pool.tile([P, 1], F32, tag="rsum", name=f"rsum{qc}")
            nc.vector.tensor_add(rsum[:], sums[:, 0:1], sums[:, 1:2])
            rinv = stat_pool.tile([P, 1], F32, tag="rinv", name=f"rinv{qc}")
            nc.vector.reciprocal(rinv[:], rsum[:])
            at_sb = at_pool.tile([P, S], BF16, tag="at", name=f"at{qc}")
            # one xbar transpose for all 16 chunks: out[p, c, q] = exp[q, c*128+p]
            nc.scalar.dma_start(
                out=at_sb.rearrange("p (c q) -> p c q", q=P),
                in_=exp_sb[:, :],
                transpose=True,
            )
            pv = psum.tile([P, D], F32, tag="ps", name=f"pv{qc}")
            pv_order = [0, 1, 2, 3, 8, 9, 10, 11, 4, 5, 6, 7, 12, 13, 14, 15]
            for i, k in enumerate(pv_order):
                for n in range(2):
                    nc.tensor.matmul(
                        pv[:, n * 512 : (n + 1) * 512],
                        lhsT=at_sb[:, k * P : (k + 1) * P],
                        rhs=vt[k][:, n * 512 : (n + 1) * 512],
                        start=(i == 0),
                        stop=(i == NS - 1),
                    )
            ot = o_pool.tile([P, D], F32, tag="o", name=f"ot{qc}")
            nc.vector.tensor_scalar_mul(ot[:], pv[:], rinv[:])
            nc.sync.dma_start(out_d[qc * P : (qc + 1) * P, :], ot[:])

        # emit scores(qc+1) before pv(qc) so the PE never stalls waiting on
        # the exp/transpose of the current chunk
        pend = scores_phase(0)
        for qc in range(NQ):
            nxt = scores_phase(qc + 1) if qc + 1 < NQ else None
            pv_phase(qc, *pend)
            pend = nxt

    nc.compile()
    return nc


def _get_nc(use_mask: bool) -> bacc.Bacc:
    if use_mask not in _NC_CACHE:
        _NC_CACHE[use_mask] = _build(use_mask)
    return _NC_CACHE[use_mask]


def kernel(x, mask, Wq, bq, Wk, bk, Wv, bv):
    x = np.asarray(x, dtype=np.float32)
    mask = np.asarray(mask, dtype=np.float32)
    Wq = np.asarray(Wq, dtype=np.float32)
    bq = np.asarray(bq, dtype=np.float32)
    Wk = np.asarray(Wk, dtype=np.float32)
    bk = np.asarray(bk, dtype=np.float32)
    Wv = np.asarray(Wv, dtype=np.float32)
    bv = np.asarray(bv, dtype=np.float32)

    B = x.shape[0]
    use_mask = bool(np.any(mask))
    nc = _get_nc(use_mask)

    bf = ml_dtypes.bfloat16
    wq_b = Wq.astype(bf)
    wk_b = Wk.astype(bf)
    wv_b = Wv.astype(bf)
    bq2 = np.ascontiguousarray(bq.reshape(ND, P).T)
    bk2 = np.ascontiguousarray(bk.reshape(ND, P).T)
    bvr = bv.reshape(1, D).astype(bf)

    in_maps = []
    for c in range(8):
        b, h = divmod(c, 2)
        off = h * Q
        xt = np.ascontiguousarray(x[b, off : off + Q].T).astype(bf)
        im = {
            "xt": xt,
            "wq": wq_b,
            "wk": wk_b,
            "wv": wv_b,
            "bq2": bq2,
            "bk2": bk2,
            "bvr": bvr,
        }
        if use_mask:
            im["maskp"] = np.ascontiguousarray(
                mask[off : off + Q] / np.float32(SCALE)
            ).astype(np.float32)
        in_maps.append(im)

    res = run_bass_kernel_spmd(nc, in_maps, core_ids=list(range(8)))

    out = np.empty((B, S, D), dtype=np.float32)
    for c in range(8):
        b, h = divmod(c, 2)
        out[b, h * Q : (h + 1) * Q, :] = res.results[c]["out"]
    return out
